# revision 6
# baseline (speedup 1.0000x reference)
"""Multi-head attention (B=4, S=2048, D=1024, H=16, d_k=64) on 8 TRN2 NeuronCores.

Sharding: batch x head-group. Core c handles batch b = c//2 and heads
[8*(c%2), 8*(c%2)+8). Each core computes Q/K/V projections for its 512
output features (column-parallel), attention for its 8 heads, and a
row-parallel partial of the W_o output projection. The host sums the two
bf16 partials per batch (the row-parallel unshard) — no collectives.

The kernel is ACT-bound: softmax needs 8*2048^2 = 33.5M exps per core at
1 elem/cycle/lane @1.2GHz = ~230us. Schedule is built so the exp stream
starts as early as possible and never starves:
- m-outer / qc-inner loop (m = head pair). Only K/Q for pair m0, V for
  pair m0 and the first token tiles are needed before the first exp;
  K/Q/V for later pairs are emitted as PE fillers inside earlier pairs'
  attention loops (one projection chunk per kb slot), one phase ahead.
- Projections are chunked (512 tokens for K/Q, 128-token x 1-pair for V)
  so the pre-attention prefix is ~12K PE cycles and DMA only needs the
  first ~2.5MB of inputs.
- W_o runs inside the m3 phase: pair (m3,qc) normalization is flushed as
  a kb6 filler of (m3,qc+1), W_o token tiles for qc at kb 9/11/13/15.
- Per-kb inner loop unchanged from the tuned baseline: scores^T via two
  concurrent row-tiled K=64 matmuls, one ACT exp (scale=1/8) per kb,
  attn@V as two M=65 matmuls (65th column of V_aug = ones accumulates
  the softmax denominators).
"""

import os
from functools import partial

import numpy as np
import ml_dtypes

import concourse.bacc as bacc
import concourse.mybir as mybir
import concourse.tile as tile
from concourse.bass_utils import run_bass_kernel_spmd

BF16 = mybir.dt.bfloat16
F32 = mybir.dt.float32
EXP = mybir.ActivationFunctionType.Exp

B, S, D = 4, 2048, 1024
H, DK = 16, 64
HPC = 8           # heads per core
FPC = HPC * DK    # 512 features per core
NP = 4            # head pairs per core
NB = 8            # din blocks of 128
NKB = 16          # key blocks of 128
NQC = 4           # q chunks of 512
QC = 512
NTT = 16          # token tiles of 128

_nc_cache = None
last_results = None


def build():
    nc = bacc.Bacc("TRN2", target_bir_lowering=False, debug=False, num_devices=8)

    xq = nc.dram_tensor("xq", [D, S], BF16, kind="ExternalInput").ap()
    xk = nc.dram_tensor("xk", [D, S], BF16, kind="ExternalInput").ap()
    xv = nc.dram_tensor("xv", [D, S], BF16, kind="ExternalInput").ap()
    wq = nc.dram_tensor("wq", [D, FPC], BF16, kind="ExternalInput").ap()
    wk = nc.dram_tensor("wk", [D, FPC], BF16, kind="ExternalInput").ap()
    wv = nc.dram_tensor("wv", [D, FPC], BF16, kind="ExternalInput").ap()
    wo = nc.dram_tensor("wo", [FPC, D], BF16, kind="ExternalInput").ap()
    mask = nc.dram_tensor("mask", [2, 128], BF16, kind="ExternalInput").ap()
    out = nc.dram_tensor("out", [S, D], BF16, kind="ExternalOutput").ap()

    with tile.TileContext(nc) as tc:
        with (
            tc.tile_pool(name="wp", bufs=1) as wp,
            tc.tile_pool(name="qkv", bufs=1) as qkv,
            tc.tile_pool(name="xp", bufs=3) as xp,
            tc.tile_pool(name="ptp", bufs=2) as ptp,
            tc.tile_pool(name="otp", bufs=4) as otp,
            tc.tile_pool(name="smalls", bufs=2) as smalls,
            tc.tile_pool(name="outp", bufs=1) as outp,
            tc.tile_pool(name="sp", bufs=2, space="PSUM") as sp,
            tc.tile_pool(name="avp", bufs=2, space="PSUM") as avp,
            tc.tile_pool(name="miscp", bufs=2, space="PSUM") as miscp,
        ):
            wq_sb = wp.tile([128, NB, NP, 128], BF16, tag="wq")
            wk_sb = wp.tile([128, NB, NP, 128], BF16, tag="wk")
            wv_sb = wp.tile([128, NB, FPC], BF16, tag="wv")
            wo_sb = wp.tile([128, NP, D], BF16, tag="wo")
            m_sb = wp.tile([2, 128], BF16, tag="mask")

            qt_sb = qkv.tile([128, NP, S], BF16, tag="qt")
            kt_sb = qkv.tile([128, NP, S], BF16, tag="kt")
            v_sb = qkv.tile([128, NKB, HPC, 65], BF16, tag="v")

            xq_sb = xp.tile([128, NB, S], BF16, tag="x", name="xq_sb")
            xk_sb = xp.tile([128, NB, S], BF16, tag="x", name="xk_sb")
            xv_sb = xp.tile([128, NB, S], BF16, tag="x", name="xv_sb")

            nc.sync.dma_start(m_sb[:], mask)
            nc.vector.memset(v_sb[:, :, :, 64], 1.0)

            # ---- DMA emission in deadline order ----
            def dma_w(w_sb_, w_, b, m):
                nc.sync.dma_start(
                    w_sb_[:, b, m], w_[b * 128:(b + 1) * 128, m * 128:(m + 1) * 128])

            def dma_x(x_sb_, x_, b, c0, c1):
                nc.sync.dma_start(
                    x_sb_[:, b, c0:c1], x_[b * 128:(b + 1) * 128, c0:c1])

            # prefix: what the first exp + first few kbs need (~2.5MB)
            for b in range(NB):
                dma_w(wk_sb, wk, b, 0)
                dma_x(xk_sb, xk, b, 0, 512)
            for b in range(NB):
                dma_w(wq_sb, wq, b, 0)
                dma_x(xq_sb, xq, b, 0, 512)
            for b in range(NB):
                nc.sync.dma_start(wv_sb[:, b, 0:128], wv[b * 128:(b + 1) * 128, 0:128])
                dma_x(xv_sb, xv, b, 0, 256)
            # streamed: xk chunks c1-3 + xv token tiles (tight deadlines)
            for b in range(NB):
                dma_x(xk_sb, xk, b, 512, 1024)
                dma_x(xv_sb, xv, b, 256, 512)
            for b in range(NB):
                dma_x(xk_sb, xk, b, 1024, 1536)
                dma_x(xv_sb, xv, b, 512, 1024)
            for b in range(NB):
                dma_x(xk_sb, xk, b, 1536, 2048)
                dma_x(xv_sb, xv, b, 1024, 2048)
            for b in range(NB):
                dma_x(xq_sb, xq, b, 512, 2048)
            for b in range(NB):
                for m in (1, 2, 3):
                    dma_w(wk_sb, wk, b, m)
                    dma_w(wq_sb, wq, b, m)
                nc.sync.dma_start(
                    wv_sb[:, b, 128:512], wv[b * 128:(b + 1) * 128, 128:512])
            for fb in range(NP):
                nc.sync.dma_start(wo_sb[:, fb], wo[fb * 128:(fb + 1) * 128, :])

            # ---- projection chunks ----
            def kq_chunk(x_sb, w_sb, dst, m, c):
                ps = miscp.tile([128, 512], F32, tag="misc", name="projc")
                for b in range(NB):
                    nc.tensor.matmul(
                        ps[:], w_sb[:, b, m], x_sb[:, b, c * 512:(c + 1) * 512],
                        start=(b == 0), stop=(b == NB - 1))
                nc.vector.tensor_copy(dst[:, m, c * 512:(c + 1) * 512], ps[:])

            def v_chunk(m, tt):
                ps = miscp.tile([128, 512], F32, tag="misc", name="vc")
                for b in range(NB):
                    nc.tensor.matmul(
                        ps[:, 0:128], xv_sb[:, b, tt * 128:(tt + 1) * 128],
                        wv_sb[:, b, m * 128:(m + 1) * 128],
                        start=(b == 0), stop=(b == NB - 1))
                nc.vector.tensor_copy(
                    v_sb[:, tt, 2 * m:2 * m + 2, 0:64],
                    ps[:, 0:128].rearrange("p (h c) -> p h c", c=64))

            # ---- attention pieces ----
            ot_tiles = {qc: otp.tile([128, NP, QC], BF16, tag="ot", name=f"ot{qc}")
                        for qc in range(NQC)}

            def finish_pair(job):
                # one-pair-delayed so the reciprocal chain is long done
                ot_t, m_t, av_sb, rec2 = job
                scp = miscp.tile([128, QC], F32, tag="misc", name="scp")
                nc.tensor.matmul(scp[:], m_sb[:], rec2[:], start=True, stop=True)
                nc.vector.tensor_mul(ot_t[0:64, m_t], av_sb[0:64, 0:QC], scp[0:64, :])
                nc.vector.tensor_mul(ot_t[64:128, m_t], av_sb[0:64, QC:2 * QC], scp[64:128, :])

            def emit_wo(qc_w, tt):
                ot_w = ot_tiles[qc_w]
                ostage = outp.tile([128, D], BF16, tag="ostage", name="ostage")
                for jc in range(2):
                    wop = miscp.tile([128, QC], F32, tag="misc", name="wop")
                    tsl = slice(tt * 128, (tt + 1) * 128)
                    for fb in range(NP):
                        nc.tensor.matmul(
                            wop[:], ot_w[:, fb, tsl], wo_sb[:, fb, jc * 512:(jc + 1) * 512],
                            start=(fb == 0), stop=(fb == NP - 1))
                    nc.vector.tensor_copy(ostage[:, jc * 512:(jc + 1) * 512], wop[:])
                row = qc_w * QC + tt * 128
                nc.sync.dma_start(out[row:row + 128, :], ostage[:])

            def attn_pair(m, qc, fillers):
                qsl = slice(qc * QC, (qc + 1) * QC)
                avA = avp.tile([128, QC], F32, tag="av", name="avA")
                avB = avp.tile([128, QC], F32, tag="av", name="avB")
                for kb in range(NKB):
                    s = sp.tile([128, 1024], F32, tag="s", name="s")
                    ksl = slice(kb * 128, (kb + 1) * 128)
                    nc.tensor.matmul(s[:, 0:512], kt_sb[0:64, m, ksl], qt_sb[0:64, m, qsl],
                                     start=True, stop=True, tile_position=(0, 0))
                    nc.tensor.matmul(s[:, 512:1024], kt_sb[64:128, m, ksl], qt_sb[64:128, m, qsl],
                                     start=True, stop=True, tile_position=(64, 0))
                    pt = ptp.tile([128, 1024], BF16, tag="pt", name="pt")
                    nc.scalar.activation(pt[:], s[:], EXP, scale=0.125)
                    nc.tensor.matmul(avA[0:65, :], v_sb[:, kb, 2 * m, 0:65], pt[:, 0:512],
                                     start=(kb == 0), stop=(kb == NKB - 1))
                    nc.tensor.matmul(avB[0:65, :], v_sb[:, kb, 2 * m + 1, 0:65], pt[:, 512:1024],
                                     start=(kb == 0), stop=(kb == NKB - 1))
                    for f in fillers.get(kb, ()):
                        f()
                av_sb = smalls.tile([128, 1024], BF16, tag="av_sb", name="av_sb")
                nc.vector.tensor_copy(av_sb[0:65, 0:QC], avA[0:65, :])
                nc.vector.tensor_copy(av_sb[0:65, QC:2 * QC], avB[0:65, :])
                den2 = smalls.tile([2, QC], BF16, tag="den2", name="den2")
                nc.sync.dma_start(den2[0:2, :], av_sb[64:65, 0:2 * QC])
                rec2 = smalls.tile([2, QC], BF16, tag="rec2", name="rec2")
                with nc.allow_low_precision(reason="bf16 softmax reciprocal, matches baseline cast"):
                    nc.vector.reciprocal(rec2[:], den2[:])
                return (ot_tiles[qc], m, av_sb, rec2)

            # ---- prefix projections: first exp after ~12K PE cycles ----
            kq_chunk(xk_sb, wk_sb, kt_sb, 0, 0)
            kq_chunk(xq_sb, wq_sb, qt_sb, 0, 0)
            v_chunk(0, 0)
            v_chunk(0, 1)

            # ---- main loop: m outer, qc inner, fillers one phase ahead ----
            pending = None
            for m in range(NP):
                for qc in range(NQC):
                    fillers = {}

                    def add(kb, fn):
                        fillers.setdefault(kb, []).append(fn)

                    if m == 0 and qc == 0:
                        for j, kb in enumerate((0, 1, 2)):
                            add(kb, partial(kq_chunk, xk_sb, wk_sb, kt_sb, 0, j + 1))
                        for j, kb in enumerate((3, 4, 5)):
                            add(kb, partial(kq_chunk, xq_sb, wq_sb, qt_sb, 0, j + 1))
                        for tt in range(2, NTT):
                            add(tt - 2, partial(v_chunk, 0, tt))
                    elif m == 0 and qc == 1:
                        for j in range(4):
                            add(2 + 4 * j, partial(kq_chunk, xk_sb, wk_sb, kt_sb, 1, j))
                    elif m == 0 and qc == 2:
                        for j in range(4):
                            add(2 + 4 * j, partial(kq_chunk, xq_sb, wq_sb, qt_sb, 1, j))
                    elif m == 0 and qc == 3:
                        for tt in range(NTT):
                            add(tt, partial(v_chunk, 1, tt))
                    elif m == 1 and qc == 0:
                        for j in range(4):
                            add(2 + 4 * j, partial(kq_chunk, xk_sb, wk_sb, kt_sb, 2, j))
                    elif m == 1 and qc == 1:
                        for j in range(4):
                            add(2 + 4 * j, partial(kq_chunk, xq_sb, wq_sb, qt_sb, 2, j))
                    elif m == 1 and qc == 2:
                        for tt in range(NTT):
                            add(tt, partial(v_chunk, 2, tt))
                    elif m == 1 and qc == 3:
                        for j in range(4):
                            add(2 + 4 * j, partial(kq_chunk, xk_sb, wk_sb, kt_sb, 3, j))
                    elif m == 2 and qc == 0:
                        for j in range(4):
                            add(2 + 4 * j, partial(kq_chunk, xq_sb, wq_sb, qt_sb, 3, j))
                    elif m == 2 and qc == 1:
                        for tt in range(NTT):
                            add(tt, partial(v_chunk, 3, tt))

                    if m == NP - 1 and pending is not None:
                        # flush mid-pair (rec2 is ready by then) so W_o for
                        # the previous q chunk can run as kb fillers here
                        pj = pending
                        pending = None
                        add(6, partial(finish_pair, pj))
                        if qc > 0:
                            for j, kb in enumerate((9, 11, 13, 15)):
                                add(kb, partial(emit_wo, qc - 1, j))

                    job = attn_pair(m, qc, fillers)
                    if pending is not None:
                        finish_pair(pending)
                    pending = job

            # drain: last pair's normalization + last q chunk's W_o
            finish_pair(pending)
            for tt in range(4):
                emit_wo(NQC - 1, tt)

    nc.compile()
    return nc


def _get_nc():
    global _nc_cache
    if _nc_cache is None:
        _nc_cache = build()
    return _nc_cache


def kernel(query, key, value, W_q, W_k, W_v, W_o):
    global last_results
    nc = _get_nc()
    bf = ml_dtypes.bfloat16

    mask = np.zeros((2, 128), bf)
    mask[0, 0:64] = 1.0
    mask[1, 64:128] = 1.0

    in_maps = []
    xt = {}
    for b in range(B):
        xt[b] = {
            "xq": np.ascontiguousarray(query[b].T).astype(bf),
            "xk": np.ascontiguousarray(key[b].T).astype(bf),
            "xv": np.ascontiguousarray(value[b].T).astype(bf),
        }
    wmaps = []
    for hg in range(2):
        r = slice(hg * FPC, (hg + 1) * FPC)
        wmaps.append({
            "wq": np.ascontiguousarray(W_q[r, :].T).astype(bf),
            "wk": np.ascontiguousarray(W_k[r, :].T).astype(bf),
            "wv": np.ascontiguousarray(W_v[r, :].T).astype(bf),
            "wo": np.ascontiguousarray(W_o[:, r].T).astype(bf),
        })
    for c in range(8):
        b, hg = c // 2, c % 2
        in_maps.append({**xt[b], **wmaps[hg], "mask": mask})

    res = run_bass_kernel_spmd(
        nc, in_maps, core_ids=list(range(8)),
        trace=bool(os.environ.get("BASS_KERNEL_TRACE")))
    last_results = res

    out = np.empty((B, S, D), np.float32)
    for b in range(B):
        out[b] = (res.results[2 * b]["out"].astype(np.float32)
                  + res.results[2 * b + 1]["out"].astype(np.float32))
    return out


# revision 9
# speedup vs baseline: 1.0721x; 1.0721x over previous
"""Multi-head attention (B=4, S=2048, D=1024, H=16, d_k=64) on 8 TRN2 NeuronCores.

Sharding: batch x head-group. Core c handles batch b = c//2 and heads
[8*(c%2), 8*(c%2)+8). Each core computes Q/K/V projections for its 512
output features (column-parallel), attention for its 8 heads, and a
row-parallel partial of the W_o output projection. The host sums the two
bf16 partials per batch (the row-parallel unshard) — no collectives.

The kernel is ACT-bound: softmax needs 8*2048^2 = 33.5M exps per core at
1 elem/cycle/lane @1.2GHz, ~1.08us per 128x1024 kb-tile, 277us total.
All engines execute their queues IN ORDER, so anything emitted between
two kb-tiles delays the exp stream by its full duration. The schedule
therefore threads every non-attention PE op (projection chunks, W_o
tiles, normalization matmuls) through a deadline-driven work queue that
releases ~0.42us micro-steps (2 N=512 matmuls or 4 N=128 matmuls) into
each kb slot — the PE slack left per exp — force-releasing whenever an
item's deadline arrives.

Loop structure: m-outer (head pair) / qc-inner. Only K/Q for pair m0 and
V for the first two token tiles precede the first exp; everything else
is queue work. W_o for chunk qc runs inside the m3 phase one qc behind
its normalization. Inner loop per kb: scores^T via two concurrent
row-tiled K=64 matmuls, one ACT exp (scale=1/8), attn@V as two M=65
matmuls (65th V_aug column = ones accumulates softmax denominators).
"""

import os
from functools import partial

import numpy as np
import ml_dtypes

import concourse.bacc as bacc
import concourse.mybir as mybir
import concourse.tile as tile
from concourse.bass_utils import run_bass_kernel_spmd

BF16 = mybir.dt.bfloat16
F32 = mybir.dt.float32
EXP = mybir.ActivationFunctionType.Exp

B, S, D = 4, 2048, 1024
H, DK = 16, 64
HPC = 8           # heads per core
FPC = HPC * DK    # 512 features per core
NP = 4            # head pairs per core
NB = 8            # din blocks of 128
NKB = 16          # key blocks of 128
NQC = 4           # q chunks of 512
QC = 512
NTT = 16          # token tiles of 128

_nc_cache = None
last_results = None


def gslot(m, qc, kb):
    return (m * NQC + qc) * NKB + kb


def build():
    nc = bacc.Bacc("TRN2", target_bir_lowering=False, debug=False, num_devices=8)

    xq = nc.dram_tensor("xq", [D, S], BF16, kind="ExternalInput").ap()
    xk = nc.dram_tensor("xk", [D, S], BF16, kind="ExternalInput").ap()
    xv = nc.dram_tensor("xv", [D, S], BF16, kind="ExternalInput").ap()
    wq = nc.dram_tensor("wq", [D, FPC], BF16, kind="ExternalInput").ap()
    wk = nc.dram_tensor("wk", [D, FPC], BF16, kind="ExternalInput").ap()
    wv = nc.dram_tensor("wv", [D, FPC], BF16, kind="ExternalInput").ap()
    wo = nc.dram_tensor("wo", [FPC, D], BF16, kind="ExternalInput").ap()
    mask = nc.dram_tensor("mask", [2, 128], BF16, kind="ExternalInput").ap()
    out = nc.dram_tensor("out", [S, D], BF16, kind="ExternalOutput").ap()

    with tile.TileContext(nc) as tc:
        with (
            tc.tile_pool(name="wp", bufs=1) as wp,
            tc.tile_pool(name="qkv", bufs=1) as qkv,
            tc.tile_pool(name="xp", bufs=3) as xp,
            tc.tile_pool(name="ptp", bufs=2) as ptp,
            tc.tile_pool(name="otp", bufs=4) as otp,
            tc.tile_pool(name="smalls", bufs=2) as smalls,
            tc.tile_pool(name="outp", bufs=2) as outp,
            tc.tile_pool(name="sp", bufs=2, space="PSUM") as sp,
            tc.tile_pool(name="avp", bufs=2, space="PSUM") as avp,
            tc.tile_pool(name="miscp", bufs=2, space="PSUM") as miscp,
        ):
            wq_sb = wp.tile([128, NB, NP, 128], BF16, tag="wq")
            wk_sb = wp.tile([128, NB, NP, 128], BF16, tag="wk")
            wv_sb = wp.tile([128, NB, FPC], BF16, tag="wv")
            wo_sb = wp.tile([128, NP, D], BF16, tag="wo")
            m_sb = wp.tile([2, 128], BF16, tag="mask")

            qt_sb = qkv.tile([128, NP, S], BF16, tag="qt")
            kt_sb = qkv.tile([128, NP, S], BF16, tag="kt")
            v_sb = qkv.tile([128, NKB, HPC, 65], BF16, tag="v")

            xq_sb = xp.tile([128, NB, S], BF16, tag="x", name="xq_sb")
            xk_sb = xp.tile([128, NB, S], BF16, tag="x", name="xk_sb")
            xv_sb = xp.tile([128, NB, S], BF16, tag="x", name="xv_sb")

            nc.sync.dma_start(m_sb[:], mask)
            nc.vector.memset(v_sb[:, :, :, 64], 1.0)

            # ---- DMA emission in deadline order ----
            def dma_w(w_sb_, w_, b, m):
                nc.sync.dma_start(
                    w_sb_[:, b, m], w_[b * 128:(b + 1) * 128, m * 128:(m + 1) * 128])

            def dma_x(x_sb_, x_, b, c0, c1):
                nc.sync.dma_start(
                    x_sb_[:, b, c0:c1], x_[b * 128:(b + 1) * 128, c0:c1])

            # scores(m0, qc0, kb0) needs all of these (~2.5MB)
            for b in range(NB):
                dma_w(wk_sb, wk, b, 0)
                dma_x(xk_sb, xk, b, 0, 512)
                dma_w(wq_sb, wq, b, 0)
                dma_x(xq_sb, xq, b, 0, 512)
            # V-m0 for the first kbs, then xk c1 (kb4), then the xv stream
            for b in range(NB):
                nc.sync.dma_start(wv_sb[:, b, 0:128], wv[b * 128:(b + 1) * 128, 0:128])
                dma_x(xv_sb, xv, b, 0, 256)
            for b in range(NB):
                dma_x(xv_sb, xv, b, 256, 512)
                dma_x(xk_sb, xk, b, 512, 1024)
            for b in range(NB):
                dma_x(xv_sb, xv, b, 512, 1024)
                dma_x(xk_sb, xk, b, 1024, 1536)
            for b in range(NB):
                dma_x(xv_sb, xv, b, 1024, 2048)
                dma_x(xk_sb, xk, b, 1536, 2048)
            for b in range(NB):
                dma_x(xq_sb, xq, b, 512, 2048)
            for b in range(NB):
                for m in (1, 2, 3):
                    dma_w(wk_sb, wk, b, m)
                    dma_w(wq_sb, wq, b, m)
                nc.sync.dma_start(
                    wv_sb[:, b, 128:512], wv[b * 128:(b + 1) * 128, 128:512])
            for fb in range(NP):
                nc.sync.dma_start(wo_sb[:, fb], wo[fb * 128:(fb + 1) * 128, :])

            # ---- micro-step emitters (misc PSUM rotates chunk-atomically) ----
            state = {}

            def kq_step(x_sb, w_sb, dst, m, c, i):
                # step i of 4: matmuls b=2i,2i+1; last step evacuates
                if i == 0:
                    state["kq"] = miscp.tile([128, 512], F32, tag="misc", name="projc")
                ps = state["kq"]
                for b in (2 * i, 2 * i + 1):
                    nc.tensor.matmul(
                        ps[:], w_sb[:, b, m], x_sb[:, b, c * 512:(c + 1) * 512],
                        start=(b == 0), stop=(b == NB - 1))
                if i == 3:
                    nc.vector.tensor_copy(dst[:, m, c * 512:(c + 1) * 512], ps[:])

            def v_step(m, tt, i):
                # step i of 2: matmuls b=4i..4i+3; last step evacuates
                if i == 0:
                    state[("v", m, tt)] = miscp.tile([128, 512], F32, tag="misc", name="vc")
                ps = state[("v", m, tt)]
                for b in range(4 * i, 4 * i + 4):
                    nc.tensor.matmul(
                        ps[:, 0:128], xv_sb[:, b, tt * 128:(tt + 1) * 128],
                        wv_sb[:, b, m * 128:(m + 1) * 128],
                        start=(b == 0), stop=(b == NB - 1))
                if i == 1:
                    nc.vector.tensor_copy(
                        v_sb[:, tt, 2 * m:2 * m + 2, 0:64],
                        ps[:, 0:128].rearrange("p (h c) -> p h c", c=64))
                    del state[("v", m, tt)]

            ot_tiles = {qc: otp.tile([128, NP, QC], BF16, tag="ot", name=f"ot{qc}")
                        for qc in range(NQC)}

            def finish_pair(job):
                ot_t, m_t, av_sb, rec2 = job
                scp = miscp.tile([128, QC], F32, tag="misc", name="scp")
                nc.tensor.matmul(scp[:], m_sb[:], rec2[:], start=True, stop=True)
                nc.vector.tensor_mul(ot_t[0:64, m_t], av_sb[0:64, 0:QC], scp[0:64, :])
                nc.vector.tensor_mul(ot_t[64:128, m_t], av_sb[0:64, QC:2 * QC], scp[64:128, :])

            def wo_step(qc_w, tt, jc, i):
                # step i of 2 for output half jc: matmuls fb=2i,2i+1; last
                # step evacuates to a half ostage and DMAs it out
                if i == 0:
                    state["wo"] = miscp.tile([128, QC], F32, tag="misc", name="wop")
                wop = state["wo"]
                ot_w = ot_tiles[qc_w]
                tsl = slice(tt * 128, (tt + 1) * 128)
                for fb in (2 * i, 2 * i + 1):
                    nc.tensor.matmul(
                        wop[:], ot_w[:, fb, tsl], wo_sb[:, fb, jc * 512:(jc + 1) * 512],
                        start=(fb == 0), stop=(fb == NP - 1))
                if i == 1:
                    ostage = outp.tile([128, QC], BF16, tag="ostage", name="ostage")
                    nc.vector.tensor_copy(ostage[:], wop[:])
                    row = qc_w * QC + tt * 128
                    nc.sync.dma_start(
                        out[row:row + 128, jc * 512:(jc + 1) * 512], ostage[:])

            # ---- deadline-driven work queue ----
            # item: (deadline_slot, cost_ns, fn). Queue is consumed FIFO; it
            # is built deadline-sorted. A kb slot pops until its ~420ns slack
            # budget is used, and keeps popping anything whose deadline is due.
            work = []

            def push(deadline, cost, fn):
                work.append([deadline, cost, fn])

            MARGIN = 2

            def push_kq(x_sb, w_sb, dst, m, c, deadline):
                for i in range(4):
                    push(deadline - (3 - i), 450, partial(kq_step, x_sb, w_sb, dst, m, c, i))

            def push_v(m, tt, deadline):
                for i in range(2):
                    push(deadline - (1 - i), 350, partial(v_step, m, tt, i))

            def drain(g, budget=470):
                spent = 0
                while work:
                    d, cost, fn = work[0]
                    if d > g + 1 and spent + cost > budget:
                        break
                    work.pop(0)
                    fn()
                    spent += cost

            # build the projection schedule:
            # K m c covers kb 4c..4c+3 of every qc -> done before (m, 0, 4c)
            # Q m c covers qc=c -> done before (m, c, 0)
            # V m tt feeds AV at kb=tt -> done before (m, 0, tt) [AV side]
            items = []
            for m in range(NP):
                for c in range(4):
                    if m == 0 and c == 0:
                        continue
                    items.append((gslot(m, 0, 4 * c) - MARGIN, "kq", xk_sb, wk_sb, kt_sb, m, c))
                for c in range(4):
                    if m == 0 and c == 0:
                        continue
                    items.append((gslot(m, c, 0) - MARGIN, "kq", xq_sb, wq_sb, qt_sb, m, c))
                for tt in range(NTT):
                    if m == 0 and tt < 2:
                        continue
                    items.append((gslot(m, 0, tt) - 1, "v", m, tt))
            items.sort(key=lambda it: it[0])
            for it in items:
                if it[1] == "kq":
                    push_kq(it[2], it[3], it[4], it[5], it[6], it[0])
                else:
                    push_v(it[2], it[3], it[0])

            # ---- attention ----
            def attn_pair(m, qc):
                qsl = slice(qc * QC, (qc + 1) * QC)
                avA = avp.tile([128, QC], F32, tag="av", name="avA")
                avB = avp.tile([128, QC], F32, tag="av", name="avB")
                first = (m == 0 and qc == 0)
                for kb in range(NKB):
                    s = sp.tile([128, 1024], F32, tag="s", name="s")
                    ksl = slice(kb * 128, (kb + 1) * 128)
                    nc.tensor.matmul(s[:, 0:512], kt_sb[0:64, m, ksl], qt_sb[0:64, m, qsl],
                                     start=True, stop=True, tile_position=(0, 0))
                    nc.tensor.matmul(s[:, 512:1024], kt_sb[64:128, m, ksl], qt_sb[64:128, m, qsl],
                                     start=True, stop=True, tile_position=(64, 0))
                    pt = ptp.tile([128, 1024], BF16, tag="pt", name="pt")
                    nc.scalar.activation(pt[:], s[:], EXP, scale=0.125)
                    if first and kb == 0:
                        # V tt0/tt1 here: after the first exp is queued, but
                        # before AV(kb0) needs them (PE runs in order)
                        for i in range(2):
                            v_step(0, 0, i)
                        for i in range(2):
                            v_step(0, 1, i)
                    nc.tensor.matmul(avA[0:65, :], v_sb[:, kb, 2 * m, 0:65], pt[:, 0:512],
                                     start=(kb == 0), stop=(kb == NKB - 1))
                    nc.tensor.matmul(avB[0:65, :], v_sb[:, kb, 2 * m + 1, 0:65], pt[:, 512:1024],
                                     start=(kb == 0), stop=(kb == NKB - 1))
                    drain(gslot(m, qc, kb))
                av_sb = smalls.tile([128, 1024], BF16, tag="av_sb", name="av_sb")
                nc.vector.tensor_copy(av_sb[0:65, 0:QC], avA[0:65, :])
                nc.vector.tensor_copy(av_sb[0:65, QC:2 * QC], avB[0:65, :])
                den2 = smalls.tile([2, QC], BF16, tag="den2", name="den2")
                nc.sync.dma_start(den2[0:2, :], av_sb[64:65, 0:2 * QC])
                rec2 = smalls.tile([2, QC], BF16, tag="rec2", name="rec2")
                with nc.allow_low_precision(reason="bf16 softmax reciprocal"):
                    nc.vector.reciprocal(rec2[:], den2[:])
                return (ot_tiles[qc], m, av_sb, rec2)

            # prefix: K m0 c0, Q m0 c0 (4 steps each, back to back)
            for i in range(4):
                kq_step(xk_sb, wk_sb, kt_sb, 0, 0, i)
            for i in range(4):
                kq_step(xq_sb, wq_sb, qt_sb, 0, 0, i)

            pending = None
            for m in range(NP):
                for qc in range(NQC):
                    if m == NP - 1:
                        # flush previous pair's norm now; queue its W_o tiles
                        # (one ~420ns step per kb slot of this pair)
                        pj = pending
                        pending = None
                        g0 = gslot(m, qc, 0)
                        # kb4: the reciprocal chain of the previous pair has
                        # ~4.4us to finish before the scp matmul consumes it
                        push(g0 + 4, 250, partial(finish_pair, pj))
                        if qc > 0:
                            for k in range(16):     # qc-1: 4 tt x 2 jc x 2 steps
                                j, jc, i = k // 4, (k // 2) % 2, k % 2
                                push(g0 + 6 + k, 450,
                                     partial(wo_step, qc - 1, j, jc, i))
                    job = attn_pair(m, qc)
                    if pending is not None:
                        finish_pair(pending)
                    pending = job

            # drain: last pair's normalization + last q chunk's W_o
            drain(10 ** 9, budget=10 ** 9)
            finish_pair(pending)
            for tt in range(4):
                for jc in range(2):
                    for i in range(2):
                        wo_step(NQC - 1, tt, jc, i)

    nc.compile()
    return nc


def _get_nc():
    global _nc_cache
    if _nc_cache is None:
        _nc_cache = build()
    return _nc_cache


def kernel(query, key, value, W_q, W_k, W_v, W_o):
    global last_results
    nc = _get_nc()
    bf = ml_dtypes.bfloat16

    mask = np.zeros((2, 128), bf)
    mask[0, 0:64] = 1.0
    mask[1, 64:128] = 1.0

    in_maps = []
    xt = {}
    for b in range(B):
        xt[b] = {
            "xq": np.ascontiguousarray(query[b].T).astype(bf),
            "xk": np.ascontiguousarray(key[b].T).astype(bf),
            "xv": np.ascontiguousarray(value[b].T).astype(bf),
        }
    wmaps = []
    for hg in range(2):
        r = slice(hg * FPC, (hg + 1) * FPC)
        wmaps.append({
            "wq": np.ascontiguousarray(W_q[r, :].T).astype(bf),
            "wk": np.ascontiguousarray(W_k[r, :].T).astype(bf),
            "wv": np.ascontiguousarray(W_v[r, :].T).astype(bf),
            "wo": np.ascontiguousarray(W_o[:, r].T).astype(bf),
        })
    for c in range(8):
        b, hg = c // 2, c % 2
        in_maps.append({**xt[b], **wmaps[hg], "mask": mask})

    res = run_bass_kernel_spmd(
        nc, in_maps, core_ids=list(range(8)),
        trace=bool(os.environ.get("BASS_KERNEL_TRACE")))
    last_results = res

    out = np.empty((B, S, D), np.float32)
    for b in range(B):
        out[b] = (res.results[2 * b]["out"].astype(np.float32)
                  + res.results[2 * b + 1]["out"].astype(np.float32))
    return out


# revision 10
# speedup vs baseline: 1.2052x; 1.1241x over previous
"""Multi-head attention (B=4, S=2048, D=1024, H=16, d_k=64) on 8 TRN2 NeuronCores.

Sharding: batch x head-group. Core c handles batch b = c//2 and heads
[8*(c%2), 8*(c%2)+8). Each core computes Q/K/V projections for its 512
output features (column-parallel), attention for its 8 heads, and a
row-parallel partial of the W_o output projection. The host sums the two
bf16 partials per batch (the row-parallel unshard) — no collectives.

The kernel is ACT-bound: softmax needs 8*2048^2 = 33.5M exps per core at
1 elem/cycle/lane @1.2GHz, ~1.08us per 128x1024 kb-tile, ~277us total.
Every engine runs its queue IN ORDER, so the schedule is built so the
ACT exp stream starts early and never waits:

- One-slot software pipeline: slot t emits exp(t), then scores(t+1),
  then attn@V(t). The next exp's input is always ready even though
  AV(t) blocks the PE until exp(t) completes.
- All non-attention PE work (projection chunks, W_o tiles, norm
  matmuls) flows through a deadline-sorted queue released in ~0.42us
  micro-steps into each slot's PE slack, force-released on deadline.
- Inputs arrive as ~16 large multi-block DMAs (one dma_start spreads
  over all 16 SDMA engines; many small DMAs serialize on ring FIFOs)
  ordered by first use: scores(0) needs only 2.5MB.
- m-outer / qc-inner loop; W_o for chunk qc runs inside the m3 phase
  one qc behind its normalization; output partials stream out bf16.

Inner loop per kb: scores^T via two concurrent row-tiled K=64 matmuls
(tile_position (0,0)/(64,0)), one ACT exp (scale=1/8), attn@V as two
M=65 matmuls (65th V_aug column = ones accumulates the softmax
denominators; max-subtraction skipped since scores ~ N(0,1)).
"""

import os
from functools import partial

import numpy as np
import ml_dtypes

import concourse.bacc as bacc
import concourse.mybir as mybir
import concourse.tile as tile
from concourse.bass_utils import run_bass_kernel_spmd

BF16 = mybir.dt.bfloat16
F32 = mybir.dt.float32
EXP = mybir.ActivationFunctionType.Exp

B, S, D = 4, 2048, 1024
H, DK = 16, 64
HPC = 8           # heads per core
FPC = HPC * DK    # 512 features per core
NP = 4            # head pairs per core
NB = 8            # din blocks of 128
NKB = 16          # key blocks of 128
NQC = 4           # q chunks of 512
QC = 512
NTT = 16          # token tiles of 128

_nc_cache = None
last_results = None


def gslot(m, qc, kb):
    return (m * NQC + qc) * NKB + kb


def build():
    nc = bacc.Bacc("TRN2", target_bir_lowering=False, debug=False, num_devices=8)

    xq = nc.dram_tensor("xq", [D, S], BF16, kind="ExternalInput").ap()
    xk = nc.dram_tensor("xk", [D, S], BF16, kind="ExternalInput").ap()
    xv = nc.dram_tensor("xv", [D, S], BF16, kind="ExternalInput").ap()
    wq = nc.dram_tensor("wq", [D, FPC], BF16, kind="ExternalInput").ap()
    wk = nc.dram_tensor("wk", [D, FPC], BF16, kind="ExternalInput").ap()
    wv = nc.dram_tensor("wv", [D, FPC], BF16, kind="ExternalInput").ap()
    wo = nc.dram_tensor("wo", [FPC, D], BF16, kind="ExternalInput").ap()
    mask = nc.dram_tensor("mask", [2, 128], BF16, kind="ExternalInput").ap()
    out = nc.dram_tensor("out", [S, D], BF16, kind="ExternalOutput").ap()

    with tile.TileContext(nc) as tc:
        with (
            tc.tile_pool(name="wp", bufs=1) as wp,
            tc.tile_pool(name="qkv", bufs=1) as qkv,
            tc.tile_pool(name="xp", bufs=3) as xp,
            tc.tile_pool(name="ptp", bufs=2) as ptp,
            tc.tile_pool(name="otp", bufs=4) as otp,
            tc.tile_pool(name="smalls", bufs=2) as smalls,
            tc.tile_pool(name="outp", bufs=2) as outp,
            tc.tile_pool(name="sp", bufs=2, space="PSUM") as sp,
            tc.tile_pool(name="avp", bufs=2, space="PSUM") as avp,
            tc.tile_pool(name="miscp", bufs=2, space="PSUM") as miscp,
        ):
            wq_sb = wp.tile([128, NB, NP, 128], BF16, tag="wq")
            wk_sb = wp.tile([128, NB, NP, 128], BF16, tag="wk")
            wv_sb = wp.tile([128, NB, FPC], BF16, tag="wv")
            wo_sb = wp.tile([128, NP, D], BF16, tag="wo")
            m_sb = wp.tile([2, 128], BF16, tag="mask")

            qt_sb = qkv.tile([128, NP, S], BF16, tag="qt")
            kt_sb = qkv.tile([128, NP, S], BF16, tag="kt")
            v_sb = qkv.tile([128, NKB, HPC, 65], BF16, tag="v")

            xq_sb = xp.tile([128, NB, S], BF16, tag="x", name="xq_sb")
            xk_sb = xp.tile([128, NB, S], BF16, tag="x", name="xk_sb")
            xv_sb = xp.tile([128, NB, S], BF16, tag="x", name="xv_sb")

            nc.sync.dma_start(m_sb[:], mask)
            nc.vector.memset(v_sb[:, :, :, 64], 1.0)

            # ---- consolidated DMAs, ordered by first use ----
            def xsrc(x_, c0, c1):
                return x_[:, c0:c1].rearrange("(b p) c -> p b c", p=128)

            nc.sync.dma_start(xk_sb[:, :, 0:512], xsrc(xk, 0, 512))
            nc.sync.dma_start(wk_sb[:, :, 0],
                              wk[:, 0:128].rearrange("(b p) c -> p b c", p=128))
            nc.sync.dma_start(xq_sb[:, :, 0:512], xsrc(xq, 0, 512))
            nc.sync.dma_start(wq_sb[:, :, 0],
                              wq[:, 0:128].rearrange("(b p) c -> p b c", p=128))
            nc.sync.dma_start(wv_sb[:, :, 0:128],
                              wv[:, 0:128].rearrange("(b p) c -> p b c", p=128))
            nc.sync.dma_start(xv_sb[:, :, 0:256], xsrc(xv, 0, 256))
            nc.sync.dma_start(xk_sb[:, :, 512:1024], xsrc(xk, 512, 1024))
            nc.sync.dma_start(xv_sb[:, :, 256:768], xsrc(xv, 256, 768))
            nc.sync.dma_start(xk_sb[:, :, 1024:2048], xsrc(xk, 1024, 2048))
            nc.sync.dma_start(xv_sb[:, :, 768:1536], xsrc(xv, 768, 1536))
            nc.sync.dma_start(xv_sb[:, :, 1536:2048], xsrc(xv, 1536, 2048))
            nc.sync.dma_start(xq_sb[:, :, 512:2048], xsrc(xq, 512, 2048))
            nc.sync.dma_start(
                wk_sb[:, :, 1:4],
                wk[:, 128:512].rearrange("(b p) (m c) -> p b m c", p=128, c=128))
            nc.sync.dma_start(
                wq_sb[:, :, 1:4],
                wq[:, 128:512].rearrange("(b p) (m c) -> p b m c", p=128, c=128))
            nc.sync.dma_start(wv_sb[:, :, 128:512],
                              wv[:, 128:512].rearrange("(b p) c -> p b c", p=128))
            nc.sync.dma_start(wo_sb[:],
                              wo.rearrange("(fb p) j -> p fb j", p=128))

            # ---- micro-step emitters (misc PSUM rotates chunk-atomically) ----
            state = {}

            def kq_step(x_sb, w_sb, dst, m, c, i):
                # step i of 4: matmuls b=2i,2i+1; last step evacuates
                if i == 0:
                    state["kq"] = miscp.tile([128, 512], F32, tag="misc", name="projc")
                ps = state["kq"]
                for b in (2 * i, 2 * i + 1):
                    nc.tensor.matmul(
                        ps[:], w_sb[:, b, m], x_sb[:, b, c * 512:(c + 1) * 512],
                        start=(b == 0), stop=(b == NB - 1))
                if i == 3:
                    nc.vector.tensor_copy(dst[:, m, c * 512:(c + 1) * 512], ps[:])

            def v_step(m, tt, i):
                # step i of 2: matmuls b=4i..4i+3; last step evacuates
                if i == 0:
                    state[("v", m, tt)] = miscp.tile([128, 512], F32, tag="misc", name="vc")
                ps = state[("v", m, tt)]
                for b in range(4 * i, 4 * i + 4):
                    nc.tensor.matmul(
                        ps[:, 0:128], xv_sb[:, b, tt * 128:(tt + 1) * 128],
                        wv_sb[:, b, m * 128:(m + 1) * 128],
                        start=(b == 0), stop=(b == NB - 1))
                if i == 1:
                    nc.vector.tensor_copy(
                        v_sb[:, tt, 2 * m:2 * m + 2, 0:64],
                        ps[:, 0:128].rearrange("p (h c) -> p h c", c=64))
                    del state[("v", m, tt)]

            ot_tiles = {qc: otp.tile([128, NP, QC], BF16, tag="ot", name=f"ot{qc}")
                        for qc in range(NQC)}

            def finish_pair(job):
                ot_t, m_t, av_sb, rec2 = job
                scp = miscp.tile([128, QC], F32, tag="misc", name="scp")
                nc.tensor.matmul(scp[:], m_sb[:], rec2[:], start=True, stop=True)
                nc.vector.tensor_mul(ot_t[0:64, m_t], av_sb[0:64, 0:QC], scp[0:64, :])
                nc.vector.tensor_mul(ot_t[64:128, m_t], av_sb[0:64, QC:2 * QC], scp[64:128, :])

            def wo_step(qc_w, tt, jc, i):
                # step i of 2 for output half jc; last step evacuates + DMAs
                if i == 0:
                    state["wo"] = miscp.tile([128, QC], F32, tag="misc", name="wop")
                wop = state["wo"]
                ot_w = ot_tiles[qc_w]
                tsl = slice(tt * 128, (tt + 1) * 128)
                for fb in (2 * i, 2 * i + 1):
                    nc.tensor.matmul(
                        wop[:], ot_w[:, fb, tsl], wo_sb[:, fb, jc * 512:(jc + 1) * 512],
                        start=(fb == 0), stop=(fb == NP - 1))
                if i == 1:
                    ostage = outp.tile([128, QC], BF16, tag="ostage", name="ostage")
                    nc.vector.tensor_copy(ostage[:], wop[:])
                    row = qc_w * QC + tt * 128
                    nc.sync.dma_start(
                        out[row:row + 128, jc * 512:(jc + 1) * 512], ostage[:])

            # ---- deadline-driven work queue (consumed FIFO, built sorted) ----
            work = []

            def push(deadline, cost, fn):
                work.append((deadline, cost, fn))

            def drain(g, budget=470):
                spent = 0
                while work:
                    d, cost, fn = work[0]
                    if d > g + 1 and spent + cost > budget:
                        break
                    work.pop(0)
                    fn()
                    spent += cost

            MARGIN = 2
            items = []
            for m in range(NP):
                for c in range(4):
                    if m == 0 and c == 0:
                        continue
                    items.append((gslot(m, 0, 4 * c) - MARGIN, "k", m, c))
                for c in range(4):
                    if m == 0 and c == 0:
                        continue
                    items.append((gslot(m, c, 0) - MARGIN, "q", m, c))
                for tt in range(NTT):
                    if m == 0 and tt < 2:
                        continue
                    items.append((gslot(m, 0, tt) - 1, "v", m, tt))
            items.sort(key=lambda it: it[0])
            for d, kind, m, x in items:
                if kind == "k":
                    for i in range(4):
                        push(d - (3 - i), 450, partial(kq_step, xk_sb, wk_sb, kt_sb, m, x, i))
                elif kind == "q":
                    for i in range(4):
                        push(d - (3 - i), 450, partial(kq_step, xq_sb, wq_sb, qt_sb, m, x, i))
                else:
                    for i in range(2):
                        push(d - (1 - i), 350, partial(v_step, m, x, i))

            # ---- prefix: K m0 c0, Q m0 c0, then scores(0) ----
            for i in range(4):
                kq_step(xk_sb, wk_sb, kt_sb, 0, 0, i)
            for i in range(4):
                kq_step(xq_sb, wq_sb, qt_sb, 0, 0, i)

            def emit_scores(m, qc, kb):
                s = sp.tile([128, 1024], F32, tag="s", name="s")
                ksl = slice(kb * 128, (kb + 1) * 128)
                qsl = slice(qc * QC, (qc + 1) * QC)
                nc.tensor.matmul(s[:, 0:512], kt_sb[0:64, m, ksl], qt_sb[0:64, m, qsl],
                                 start=True, stop=True, tile_position=(0, 0))
                nc.tensor.matmul(s[:, 512:1024], kt_sb[64:128, m, ksl], qt_sb[64:128, m, qsl],
                                 start=True, stop=True, tile_position=(64, 0))
                return s

            SLOTS = [(m, qc, kb) for m in range(NP) for qc in range(NQC)
                     for kb in range(NKB)]
            s_cur = emit_scores(0, 0, 0)
            pending = None
            avA = avB = None
            for t, (m, qc, kb) in enumerate(SLOTS):
                pt = ptp.tile([128, 1024], BF16, tag="pt", name="pt")
                nc.scalar.activation(pt[:], s_cur[:], EXP, scale=0.125)
                if t + 1 < len(SLOTS):
                    s_cur = emit_scores(*SLOTS[t + 1])
                if kb == 0:
                    avA = avp.tile([128, QC], F32, tag="av", name="avA")
                    avB = avp.tile([128, QC], F32, tag="av", name="avB")
                    if m == NP - 1:
                        # flush the previous pair's norm mid-pair (kb4: its
                        # reciprocal chain has ~4us to finish) and queue the
                        # previous q chunk's W_o tiles one step per slot
                        pj = pending
                        pending = None
                        g0 = gslot(m, qc, 0)
                        push(g0 + 4, 250, partial(finish_pair, pj))
                        if qc > 0:
                            for k in range(16):
                                j, jc, i = k // 4, (k // 2) % 2, k % 2
                                push(g0 + 6 + k, 450,
                                     partial(wo_step, qc - 1, j, jc, i))
                if t == 0:
                    # V tt0/tt1 land between the first exp and AV(0)
                    for i in range(2):
                        v_step(0, 0, i)
                    for i in range(2):
                        v_step(0, 1, i)
                nc.tensor.matmul(avA[0:65, :], v_sb[:, kb, 2 * m, 0:65], pt[:, 0:512],
                                 start=(kb == 0), stop=(kb == NKB - 1))
                nc.tensor.matmul(avB[0:65, :], v_sb[:, kb, 2 * m + 1, 0:65], pt[:, 512:1024],
                                 start=(kb == 0), stop=(kb == NKB - 1))
                if kb == NKB - 1:
                    av_sb = smalls.tile([128, 1024], BF16, tag="av_sb", name="av_sb")
                    nc.vector.tensor_copy(av_sb[0:65, 0:QC], avA[0:65, :])
                    nc.vector.tensor_copy(av_sb[0:65, QC:2 * QC], avB[0:65, :])
                    den2 = smalls.tile([2, QC], BF16, tag="den2", name="den2")
                    nc.sync.dma_start(den2[0:2, :], av_sb[64:65, 0:2 * QC])
                    rec2 = smalls.tile([2, QC], BF16, tag="rec2", name="rec2")
                    with nc.allow_low_precision(reason="bf16 softmax reciprocal"):
                        nc.vector.reciprocal(rec2[:], den2[:])
                    job = (ot_tiles[qc], m, av_sb, rec2)
                    if m < NP - 1:
                        if pending is not None:
                            finish_pair(pending)
                        pending = job
                    else:
                        pending = job
                drain(t)

            # drain: last pair's normalization + last q chunk's W_o
            drain(10 ** 9, budget=10 ** 9)
            finish_pair(pending)
            for tt in range(4):
                for jc in range(2):
                    for i in range(2):
                        wo_step(NQC - 1, tt, jc, i)

    nc.compile()
    return nc


def _get_nc():
    global _nc_cache
    if _nc_cache is None:
        _nc_cache = build()
    return _nc_cache


def kernel(query, key, value, W_q, W_k, W_v, W_o):
    global last_results
    nc = _get_nc()
    bf = ml_dtypes.bfloat16

    mask = np.zeros((2, 128), bf)
    mask[0, 0:64] = 1.0
    mask[1, 64:128] = 1.0

    in_maps = []
    xt = {}
    for b in range(B):
        xt[b] = {
            "xq": np.ascontiguousarray(query[b].T).astype(bf),
            "xk": np.ascontiguousarray(key[b].T).astype(bf),
            "xv": np.ascontiguousarray(value[b].T).astype(bf),
        }
    wmaps = []
    for hg in range(2):
        r = slice(hg * FPC, (hg + 1) * FPC)
        wmaps.append({
            "wq": np.ascontiguousarray(W_q[r, :].T).astype(bf),
            "wk": np.ascontiguousarray(W_k[r, :].T).astype(bf),
            "wv": np.ascontiguousarray(W_v[r, :].T).astype(bf),
            "wo": np.ascontiguousarray(W_o[:, r].T).astype(bf),
        })
    for c in range(8):
        b, hg = c // 2, c % 2
        in_maps.append({**xt[b], **wmaps[hg], "mask": mask})

    res = run_bass_kernel_spmd(
        nc, in_maps, core_ids=list(range(8)),
        trace=bool(os.environ.get("BASS_KERNEL_TRACE")))
    last_results = res

    out = np.empty((B, S, D), np.float32)
    for b in range(B):
        out[b] = (res.results[2 * b]["out"].astype(np.float32)
                  + res.results[2 * b + 1]["out"].astype(np.float32))
    return out
